# revision 3
# baseline (speedup 1.0000x reference)
"""Trainium2 Bass kernel for single-head causal attention (B=4, T=2048, C=2048).

Algebraic folding: y = softmax(mask(x Wq^T Wk x^T / sqrt(C))) x (Wo Wv)^T, so
the host folds the weight-only products M = Wq^T @ Wk and Wvo = Wo @ Wv once
(exact fp32), and the device computes Q' = x @ M, scores = Q' @ x^T, then
y = attn @ x @ Wvo^T. This removes the K and V projections and both K/V
AllGathers: no collectives at all, and per-core PE work drops by ~2/5.

Sharding: 8 cores = 4 batches x 2 row-interleave. The two cores of a batch
split the 16 128-row blocks so causal work balances exactly under one SPMD
program: slot i (i=0..7) owns global block r = 2i + sigma with sigma
alternating between the cores; slot i always processes 2i+2 128-col score
chunks, so the instruction stream is h-independent and the host-built mask
absorbs the causal boundary (including one fully-masked wasted chunk on the
even-sigma core).

All matmuls bf16 (separate LDWEIGHTS pipelines ahead via the PE reorder
window + FWL), fp32 PSUM accumulation. Attention runs in the transposed
domain (scores.T = [s, t]); softmax denominators via ones-matmul partition
reduction, folded into the final projection as a per-row scale. Q'.T, z.T
and x (AV stationary) stay SBUF-resident; x^T streams per 256-col chunk with
one stationary load serving every slot's scores matmul.
"""
import sys

sys.path.insert(0, "/opt/trn_rl_repo")
import numpy as np

_CACHE = {}

B = 4
T_FULL = 2048
C_FULL = 2048
NEG = -1e30


def _build(T_, C_, reps=1):
    import concourse.bacc as bacc
    import concourse.mybir as mybir
    import concourse.tile as tile

    F32 = mybir.dt.float32
    BF16 = mybir.dt.bfloat16
    AF = mybir.ActivationFunctionType
    SCALE = 1.0 / float(np.sqrt(C_FULL))

    CC = C_ // 128      # contraction 128-chunks
    ND = C_ // 512      # 512-wide d / e chunks
    TOWN = T_ // 2      # owned rows per core
    NSLOT = T_ // 256   # owned 128-row blocks (slots) per core
    NM = T_ // 256      # 256-s chunks of the full sequence

    nc = bacc.Bacc("TRN2", target_bir_lowering=False, debug=False, num_devices=8)
    xTq_d = nc.declare_dram_parameter("xTq", [C_, TOWN], BF16, isOutput=False)
    xT_d = nc.declare_dram_parameter("xT", [C_, T_], BF16, isOutput=False)
    xN_d = nc.declare_dram_parameter("xN", [T_, C_], BF16, isOutput=False)
    M_d = nc.declare_dram_parameter("M", [C_, C_], BF16, isOutput=False)
    WvoT_d = nc.declare_dram_parameter("WvoT", [C_, C_], BF16, isOutput=False)
    mb_d = nc.declare_dram_parameter("mb", [NSLOT, 2, 128, 128], F32, isOutput=False)
    ones_d = nc.declare_dram_parameter("ones", [128, 2], BF16, isOutput=False)
    y_d = nc.declare_dram_parameter("y", [TOWN, C_], F32, isOutput=True)

    with tile.TileContext(nc) as tc:
        with (
            tc.tile_pool(name="stage", bufs=4) as stage,
            tc.tile_pool(name="cst", bufs=1) as cst,
        ):
            for _rep in range(reps):
                onest = cst.tile([128, 2], BF16, tag="ones")
                recipt = cst.tile([128, NSLOT], F32, tag="recip")
                mbt = cst.tile([128, NSLOT, 2, 128], F32, tag="mb")
                QT = cst.tile([128, CC, TOWN], BF16, tag="qtr")
                OT = cst.tile([128, CC, TOWN], BF16, tag="otr")

                # pkt lives outside the P1 space so the score-chunk prefetch
                # DMAs are not WAR-gated on P1 reads; the issue points are
                # interleaved between P1's weight loads below so they don't
                # delay the chunks P1 needs first
                pkt_cm = tc.tile_pool(name="pkt", bufs=3)
                pkt = pkt_cm.__enter__()
                ktp_pre = {}

                def _prefetch_kt(m):
                    kt = pkt.tile([128, CC, 256], BF16, tag="kt", name=f"kt{m}")
                    nc.sync.dma_start(
                        kt[:],
                        xT_d[:, 256 * m:256 * m + 256].rearrange(
                            "(cc p) s -> p cc s", p=128
                        ),
                    )
                    ktp_pre[m] = kt

                wo0 = pkt.tile([128, CC, 512], BF16, tag="w0", bufs=1)

                # ======== P1: Q' = x @ M for owned rows -> QT (SBUF) ========
                with tc.tile_pool(name="px", bufs=1) as px:
                    xtq = px.tile([128, CC, TOWN], BF16, tag="xtq")
                    with (
                        tc.tile_pool(name="pwq", bufs=3) as pwq,
                        tc.tile_pool(name="ps3", bufs=8, space="PSUM") as ps3,
                    ):
                        # first weight chunk issued before xtq so the PE can
                        # start as soon as (wq cc0-3, xtq c=0) land
                        # interleave the first weight chunk with xtq so the PE
                        # streams c-chunks just behind the DMA
                        wq0 = pwq.tile([128, CC, 512], BF16, tag="w", name="wq0")
                        nc.sync.dma_start(
                            wq0[:, 0:4, :],
                            M_d[0:512, 0:512].rearrange("(cc p) d -> p cc d", p=128),
                        )
                        for c in range(CC):
                            nc.sync.dma_start(
                                xtq[:, c, :], xTq_d[128 * c:128 * c + 128, :]
                            )
                            if c in (2, 6, 10):
                                q4 = {2: 1, 6: 2, 10: 3}[c]
                                nc.sync.dma_start(
                                    wq0[:, 4 * q4:4 * q4 + 4, :],
                                    M_d[512 * q4:512 * q4 + 512, 0:512].rearrange(
                                        "(cc p) d -> p cc d", p=128
                                    ),
                                )
                        nc.sync.dma_start(onest[:], ones_d[:])
                        nc.sync.dma_start(
                            mbt[:], mb_d[:].rearrange("ns k p n -> p ns k n")
                        )
                        _prefetch_kt(0)
                        for dd in range(ND):
                            if dd == 0:
                                wq = wq0
                            else:
                                wq = pwq.tile([128, CC, 512], BF16, tag="w")
                                for q4 in range(4):
                                    nc.sync.dma_start(
                                        wq[:, 4 * q4:4 * q4 + 4, :],
                                        M_d[
                                            512 * q4:512 * q4 + 512,
                                            512 * dd:512 * dd + 512,
                                        ].rearrange("(cc p) d -> p cc d", p=128),
                                    )
                                if dd <= 2:
                                    _prefetch_kt(dd)
                            qps = [
                                ps3.tile([128, 512], F32, tag="ps3", name=f"qps{dd}_{g}")
                                for g in range(8)
                            ]
                            for c in range(CC):
                                for d4 in range(4):
                                    for tt in range(2):
                                        nc.tensor.matmul(
                                            qps[2 * d4 + tt][:],
                                            wq[:, c, 128 * d4:128 * d4 + 128],
                                            xtq[:, c, 512 * tt:512 * tt + 512],
                                            start=(c == 0),
                                            stop=(c == CC - 1),
                                        )
                            for d4 in range(4):
                                d = 4 * dd + d4
                                for tt in range(2):
                                    nc.scalar.activation(
                                        QT[:, d, 512 * tt:512 * tt + 512],
                                        qps[2 * d4 + tt][:],
                                        AF.Copy,
                                    )

                # ======== P2: attention (transposed domain) ========
                with (
                    tc.tile_pool(name="pv", bufs=1) as pv,
                    tc.tile_pool(name="pattn", bufs=NSLOT * (NSLOT + 1)) as pattn,
                ):
                    attn = {
                        i: [
                            pattn.tile([128, 128], BF16, tag="attn", name=f"at{i}_{k}")
                            for k in range(2 * i + 2)
                        ]
                        for i in range(NSLOT)
                    }
                    # scores sweep: one stationary x^T chunk serves all slots;
                    # up to 4 slots' [128,128] scores pack one PSUM bank
                    with tc.tile_pool(name="pssc", bufs=4, space="PSUM") as pssc:
                        for m in range(NM):
                            if m < 3:
                                ktp = ktp_pre[m]
                            else:
                                ktp = pkt.tile(
                                    [128, CC, 256], BF16, tag="kt", name=f"kt{m}"
                                )
                                nc.sync.dma_start(
                                    ktp[:],
                                    xT_d[:, 256 * m:256 * m + 256].rearrange(
                                        "(cc p) s -> p cc s", p=128
                                    ),
                                )
                            nsl = NSLOT - m
                            for sub in range(2):
                                k = 2 * m + sub
                                spsg = [
                                    pssc.tile(
                                        [128, 512], F32, tag="sps", name=f"sps{k}_{g}"
                                    )
                                    for g in range((nsl + 3) // 4)
                                ]
                                for g in range((nsl + 3) // 4):
                                    nc.vector.memset(spsg[g][:], 0.0)
                                sl = lambda ii: spsg[ii // 4][
                                    :, 128 * (ii % 4):128 * (ii % 4) + 128
                                ]
                                for d in range(CC):
                                    for ii in range(nsl):
                                        nc.tensor.matmul(
                                            sl(ii),
                                            ktp[:, d, 128 * sub:128 * sub + 128],
                                            QT[:, d, 128 * (m + ii):128 * (m + ii) + 128],
                                            start=False,
                                            stop=(d == CC - 1),
                                            skip_group_check=True,
                                        )
                                for ii in range(nsl):
                                    i = m + ii
                                    if i == m:
                                        nc.vector.tensor_add(
                                            sl(ii), sl(ii), mbt[:, i, sub, :]
                                        )
                                    nc.scalar.activation(
                                        attn[i][k][:], sl(ii), AF.Exp, scale=SCALE
                                    )
                    X_sb = pv.tile([128, T_ // 128, C_], BF16, tag="xsb")
                    for k in range(T_ // 128):
                        nc.sync.dma_start(X_sb[:, k, :], xN_d[128 * k:128 * k + 128, :])
                    for q4 in range(4):
                        nc.sync.dma_start(
                            wo0[:, 4 * q4:4 * q4 + 4, :],
                            WvoT_d[512 * q4:512 * q4 + 512, 0:512].rearrange(
                                "(cc p) d -> p cc d", p=128
                            ),
                        )
                    # softmax denominators -> 1/denom per owned row
                    with tc.tile_pool(name="psr", bufs=4, space="PSUM") as psr:
                        for i in range(NSLOT):
                            rps = psr.tile([128, 2], F32, tag="rps", name=f"rps{i}")
                            for k in range(2 * i + 2):
                                nc.tensor.matmul(
                                    rps[:],
                                    attn[i][k][:],
                                    onest[:],
                                    start=(k == 0),
                                    stop=(k == 2 * i + 1),
                                )
                            nc.vector.reciprocal(recipt[:, i:i + 1], rps[:, 0:1])
                    # z.T = x.T @ attn accumulated per slot -> OT (SBUF);
                    # 4 d-chunks of [128,128] pack one PSUM bank
                    with tc.tile_pool(name="psav", bufs=8, space="PSUM") as psav:
                        for i in range(NSLOT):
                            avg = [
                                psav.tile([128, 512], F32, tag="av", name=f"av{i}_{g}")
                                for g in range(4)
                            ]
                            for g in range(4):
                                nc.vector.memset(avg[g][:], 0.0)
                            # d-outer so each d-chunk's PSUM completes early and
                            # its OT copy overlaps the remaining AV matmuls
                            for d in range(CC):
                                src = avg[d // 4][:, 128 * (d % 4):128 * (d % 4) + 128]
                                for k in range(2 * i + 2):
                                    nc.tensor.matmul(
                                        src,
                                        X_sb[:, k, 128 * d:128 * d + 128],
                                        attn[i][k][:],
                                        start=False,
                                        stop=(k == 2 * i + 1),
                                        skip_group_check=True,
                                    )
                                if d % 2 == 0:
                                    nc.vector.tensor_copy(
                                        OT[:, d, 128 * i:128 * i + 128], src
                                    )
                                else:
                                    nc.scalar.activation(
                                        OT[:, d, 128 * i:128 * i + 128], src, AF.Copy
                                    )

                # ======== P3: y = (z.T.T @ WvoT) * recip ========
                with (
                    tc.tile_pool(name="pwo", bufs=2) as pwo,
                    tc.tile_pool(name="psf", bufs=4, space="PSUM") as psf,
                ):
                    for e in range(ND):
                        if e == 0:
                            wo = wo0
                        else:
                            wo = pwo.tile([128, CC, 512], BF16, tag="w")
                            for q4 in range(4):
                                nc.sync.dma_start(
                                    wo[:, 4 * q4:4 * q4 + 4, :],
                                    WvoT_d[
                                        512 * q4:512 * q4 + 512, 512 * e:512 * e + 512
                                    ].rearrange("(cc p) d -> p cc d", p=128),
                                )
                        for i in range(NSLOT):
                            fps = psf.tile([128, 512], F32, tag="fps")
                            for d in range(CC):
                                nc.tensor.matmul(
                                    fps[:],
                                    OT[:, d, 128 * i:128 * i + 128],
                                    wo[:, d, :],
                                    start=(d == 0),
                                    stop=(d == CC - 1),
                                )
                            yt = stage.tile([128, 512], F32, tag="yt")
                            nc.vector.tensor_scalar_mul(
                                yt[:], fps[:], recipt[:, i:i + 1]
                            )
                            nc.sync.dma_start(
                                y_d[128 * i:128 * i + 128, 512 * e:512 * e + 512],
                                yt[:],
                            )
                pkt_cm.__exit__(None, None, None)
    nc.compile()
    return nc


def _own_blocks(h, nslot):
    return [2 * i + (i % 2 if h == 0 else 1 - i % 2) for i in range(nslot)]


def _host_prep(x, Wq, Wk, Wv, Wo, T_, C_):
    import ml_dtypes

    bf16 = ml_dtypes.bfloat16
    NSLOT = T_ // 256
    x = np.asarray(x, np.float32)
    Wq = np.asarray(Wq, np.float32)
    Wk = np.asarray(Wk, np.float32)
    Wv = np.asarray(Wv, np.float32)
    Wo = np.asarray(Wo, np.float32)
    M = np.ascontiguousarray(Wq.T @ Wk).astype(bf16)
    WvoT = np.ascontiguousarray((Wo @ Wv).T).astype(bf16)
    ones = np.ones((128, 2), bf16)
    masks = {}
    for h in (0, 1):
        own = _own_blocks(h, NSLOT)
        mb = np.zeros((NSLOT, 2, 128, 128), np.float32)
        for i, r in enumerate(own):
            ci = 2 * i + 2
            for idx in range(2):
                k = ci - 2 + idx
                s_idx = 128 * k + np.arange(128)[:, None]
                t_idx = 128 * r + np.arange(128)[None, :]
                mb[i, idx] = np.where(s_idx <= t_idx, 0.0, NEG)
        masks[h] = mb
    xN_b, xT_b = {}, {}
    for b in range(min(B, x.shape[0])):
        xN_b[b] = np.ascontiguousarray(x[b]).astype(bf16)
        xT_b[b] = np.ascontiguousarray(x[b].T).astype(bf16)
    in_maps = []
    for core in range(8):
        b, h = core // 2, core % 2
        b = b % x.shape[0]
        xN = xN_b[b]
        xT = xT_b[b]
        own = _own_blocks(h, NSLOT)
        own_cols = np.concatenate([np.arange(128 * r, 128 * r + 128) for r in own])
        xTq = np.ascontiguousarray(xT[:, own_cols])
        in_maps.append(
            {
                "xTq": xTq,
                "xT": xT,
                "xN": xN,
                "M": M,
                "WvoT": WvoT,
                "mb": masks[h],
                "ones": ones,
            }
        )
    return in_maps, masks


def kernel(x, Wq, Wk, Wv, Wo):
    from concourse.bass_utils import run_bass_kernel_spmd

    T_, C_ = T_FULL, C_FULL
    key = (T_, C_)
    if key not in _CACHE:
        _CACHE[key] = _build(T_, C_)
    nc = _CACHE[key]
    in_maps, _ = _host_prep(x, Wq, Wk, Wv, Wo, T_, C_)
    res = run_bass_kernel_spmd(nc, in_maps, list(range(8)))
    NSLOT = T_ // 256
    y = np.zeros((B, T_, C_), np.float32)
    for core in range(8):
        b, h = core // 2, core % 2
        yc = res.results[core]["y"]
        for i, r in enumerate(_own_blocks(h, NSLOT)):
            y[b, 128 * r:128 * r + 128, :] = yc[128 * i:128 * i + 128, :]
    return y


# revision 5
# speedup vs baseline: 1.0328x; 1.0328x over previous
"""Trainium2 Bass kernel for single-head causal attention (B=4, T=2048, C=2048).

Algebraic folding: y = softmax(mask(x Wq^T Wk x^T / sqrt(C))) x (Wo Wv)^T, so
the host folds the weight-only products M = Wq^T @ Wk and Wvo = Wo @ Wv once
(exact fp32), and the device computes Q' = x @ M, scores = Q' @ x^T, then
y = attn @ x @ Wvo^T. This removes the K and V projections and both K/V
AllGathers: no collectives at all, and per-core PE work drops by ~2/5.

Sharding: 8 cores = 4 batches x 2 row-interleave. The two cores of a batch
split the 16 128-row blocks so causal work balances exactly under one SPMD
program: slot i (i=0..7) owns global block r = 2i + sigma with sigma
alternating between the cores; slot i always processes 2i+2 128-col score
chunks, so the instruction stream is h-independent and the host-built mask
absorbs the causal boundary (including one fully-masked wasted chunk on the
even-sigma core).

All matmuls bf16 (separate LDWEIGHTS pipelines ahead via the PE reorder
window + FWL), fp32 PSUM accumulation. Attention runs in the transposed
domain (scores.T = [s, t]); softmax denominators via ones-matmul partition
reduction, folded into the final projection as a per-row scale. Q'.T, z.T
and x (AV stationary) stay SBUF-resident; x^T streams per 256-col chunk with
one stationary load serving every slot's scores matmul.
"""
import sys

sys.path.insert(0, "/opt/trn_rl_repo")
import numpy as np

_CACHE = {}

B = 4
T_FULL = 2048
C_FULL = 2048
NEG = -1e30


def _build(T_, C_, reps=1):
    import concourse.bacc as bacc
    import concourse.mybir as mybir
    import concourse.tile as tile

    F32 = mybir.dt.float32
    BF16 = mybir.dt.bfloat16
    AF = mybir.ActivationFunctionType
    SCALE = 1.0 / float(np.sqrt(C_FULL))

    CC = C_ // 128      # contraction 128-chunks
    ND = C_ // 512      # 512-wide d / e chunks
    TOWN = T_ // 2      # owned rows per core
    NSLOT = T_ // 256   # owned 128-row blocks (slots) per core
    NM = T_ // 256      # 256-s chunks of the full sequence

    nc = bacc.Bacc("TRN2", target_bir_lowering=False, debug=False, num_devices=8)
    xTq_d = nc.declare_dram_parameter("xTq", [C_, TOWN], BF16, isOutput=False)
    xT_d = nc.declare_dram_parameter("xT", [C_, T_], BF16, isOutput=False)
    xN_d = nc.declare_dram_parameter("xN", [T_, C_], BF16, isOutput=False)
    M_d = nc.declare_dram_parameter("M", [C_, C_], BF16, isOutput=False)
    WvoT_d = nc.declare_dram_parameter("WvoT", [C_, C_], BF16, isOutput=False)
    mb_d = nc.declare_dram_parameter("mb", [NSLOT, 2, 128, 128], F32, isOutput=False)
    ones_d = nc.declare_dram_parameter("ones", [128, 2], BF16, isOutput=False)
    y_d = nc.declare_dram_parameter("y", [TOWN, C_], F32, isOutput=True)

    with tile.TileContext(nc) as tc:
        with (
            tc.tile_pool(name="stage", bufs=4) as stage,
            tc.tile_pool(name="cst", bufs=1) as cst,
        ):
            for _rep in range(reps):
                onest = cst.tile([128, 2], BF16, tag="ones")
                recipt = cst.tile([128, NSLOT], F32, tag="recip")
                mbt = cst.tile([128, NSLOT, 2, 128], F32, tag="mb")
                QT = cst.tile([128, CC, TOWN], BF16, tag="qtr")
                OT = cst.tile([128, CC, TOWN], BF16, tag="otr")

                # pkt lives outside the P1 space so the score-chunk prefetch
                # DMAs are not WAR-gated on P1 reads; the issue points are
                # interleaved between P1's weight loads below so they don't
                # delay the chunks P1 needs first
                pkt_cm = tc.tile_pool(name="pkt", bufs=3)
                pkt = pkt_cm.__enter__()
                ktp_pre = {}

                def _prefetch_kt(m):
                    kt = pkt.tile([128, CC, 256], BF16, tag="kt", name=f"kt{m}")
                    nc.sync.dma_start(
                        kt[:],
                        xT_d[:, 256 * m:256 * m + 256].rearrange(
                            "(cc p) s -> p cc s", p=128
                        ),
                    )
                    ktp_pre[m] = kt

                wo0 = pkt.tile([128, CC, 512], BF16, tag="w0", bufs=1)

                # ======== P1: Q' = x @ M for owned rows -> QT (SBUF) ========
                with tc.tile_pool(name="px", bufs=1) as px:
                    xtq = px.tile([128, CC, TOWN], BF16, tag="xtq")
                    with (
                        tc.tile_pool(name="pwq", bufs=3) as pwq,
                        tc.tile_pool(name="ps3", bufs=8, space="PSUM") as ps3,
                    ):
                        # first weight chunk issued before xtq so the PE can
                        # start as soon as (wq cc0-3, xtq c=0) land
                        # interleave the first weight chunk with xtq so the PE
                        # streams c-chunks just behind the DMA
                        wq0 = pwq.tile([128, CC, 512], BF16, tag="w", name="wq0")
                        nc.sync.dma_start(
                            wq0[:, 0:4, :],
                            M_d[0:512, 0:512].rearrange("(cc p) d -> p cc d", p=128),
                        )
                        for c in range(CC):
                            nc.sync.dma_start(
                                xtq[:, c, :], xTq_d[128 * c:128 * c + 128, :]
                            )
                            if c in (2, 6, 10):
                                q4 = {2: 1, 6: 2, 10: 3}[c]
                                nc.sync.dma_start(
                                    wq0[:, 4 * q4:4 * q4 + 4, :],
                                    M_d[512 * q4:512 * q4 + 512, 0:512].rearrange(
                                        "(cc p) d -> p cc d", p=128
                                    ),
                                )
                        nc.sync.dma_start(onest[:], ones_d[:])
                        nc.sync.dma_start(
                            mbt[:], mb_d[:].rearrange("ns k p n -> p ns k n")
                        )
                        _prefetch_kt(0)
                        for dd in range(ND):
                            if dd == 0:
                                wq = wq0
                            else:
                                wq = pwq.tile([128, CC, 512], BF16, tag="w")
                                for q4 in range(4):
                                    nc.sync.dma_start(
                                        wq[:, 4 * q4:4 * q4 + 4, :],
                                        M_d[
                                            512 * q4:512 * q4 + 512,
                                            512 * dd:512 * dd + 512,
                                        ].rearrange("(cc p) d -> p cc d", p=128),
                                    )
                                if dd <= 2:
                                    _prefetch_kt(dd)
                            qps = [
                                ps3.tile([128, 512], F32, tag="ps3", name=f"qps{dd}_{g}")
                                for g in range(8)
                            ]
                            for c in range(CC):
                                for d4 in range(4):
                                    for tt in range(2):
                                        nc.tensor.matmul(
                                            qps[2 * d4 + tt][:],
                                            wq[:, c, 128 * d4:128 * d4 + 128],
                                            xtq[:, c, 512 * tt:512 * tt + 512],
                                            start=(c == 0),
                                            stop=(c == CC - 1),
                                        )
                            for d4 in range(4):
                                d = 4 * dd + d4
                                for tt in range(2):
                                    if tt == 0:
                                        nc.scalar.activation(
                                            QT[:, d, 512 * tt:512 * tt + 512],
                                            qps[2 * d4 + tt][:],
                                            AF.Copy,
                                        )
                                    else:
                                        nc.vector.tensor_copy(
                                            QT[:, d, 512 * tt:512 * tt + 512],
                                            qps[2 * d4 + tt][:],
                                        )

                # ======== P2: attention (transposed domain) ========
                with (
                    tc.tile_pool(name="pv", bufs=1) as pv,
                    tc.tile_pool(name="pattn", bufs=NSLOT * (NSLOT + 1)) as pattn,
                ):
                    attn = {
                        i: [
                            pattn.tile([128, 128], BF16, tag="attn", name=f"at{i}_{k}")
                            for k in range(2 * i + 2)
                        ]
                        for i in range(NSLOT)
                    }
                    # scores sweep: one stationary x^T chunk serves all slots;
                    # up to 4 slots' [128,128] scores pack one PSUM bank
                    with tc.tile_pool(name="pssc", bufs=4, space="PSUM") as pssc:
                        for m in range(NM):
                            if m < 3:
                                ktp = ktp_pre[m]
                            else:
                                ktp = pkt.tile(
                                    [128, CC, 256], BF16, tag="kt", name=f"kt{m}"
                                )
                                nc.sync.dma_start(
                                    ktp[:],
                                    xT_d[:, 256 * m:256 * m + 256].rearrange(
                                        "(cc p) s -> p cc s", p=128
                                    ),
                                )
                            nsl = NSLOT - m
                            for sub in range(2):
                                k = 2 * m + sub
                                spsg = [
                                    pssc.tile(
                                        [128, 512], F32, tag="sps", name=f"sps{k}_{g}"
                                    )
                                    for g in range((nsl + 3) // 4)
                                ]
                                for g in range((nsl + 3) // 4):
                                    nc.vector.memset(spsg[g][:], 0.0)
                                sl = lambda ii: spsg[ii // 4][
                                    :, 128 * (ii % 4):128 * (ii % 4) + 128
                                ]
                                # live slots' QT columns are contiguous, and so
                                # is each 4-slot PSUM group: one wide matmul per
                                # group instead of 4 narrow ones
                                for d in range(CC):
                                    for g in range((nsl + 3) // 4):
                                        cw = min(512, 128 * (nsl - 4 * g))
                                        nc.tensor.matmul(
                                            spsg[g][:, 0:cw],
                                            ktp[:, d, 128 * sub:128 * sub + 128],
                                            QT[
                                                :, d,
                                                128 * (m + 4 * g):128 * (m + 4 * g) + cw,
                                            ],
                                            start=False,
                                            stop=(d == CC - 1),
                                            skip_group_check=True,
                                        )
                                for ii in range(nsl):
                                    i = m + ii
                                    if i == m:
                                        nc.vector.tensor_add(
                                            sl(ii), sl(ii), mbt[:, i, sub, :]
                                        )
                                    nc.scalar.activation(
                                        attn[i][k][:], sl(ii), AF.Exp, scale=SCALE
                                    )
                    X_sb = pv.tile([128, T_ // 128, C_], BF16, tag="xsb")
                    for k in range(T_ // 128):
                        nc.sync.dma_start(X_sb[:, k, :], xN_d[128 * k:128 * k + 128, :])
                    for q4 in range(4):
                        nc.sync.dma_start(
                            wo0[:, 4 * q4:4 * q4 + 4, :],
                            WvoT_d[512 * q4:512 * q4 + 512, 0:512].rearrange(
                                "(cc p) d -> p cc d", p=128
                            ),
                        )
                    # softmax denominators -> 1/denom per owned row
                    with tc.tile_pool(name="psr", bufs=4, space="PSUM") as psr:
                        for i in range(NSLOT):
                            rps = psr.tile([128, 2], F32, tag="rps", name=f"rps{i}")
                            for k in range(2 * i + 2):
                                nc.tensor.matmul(
                                    rps[:],
                                    attn[i][k][:],
                                    onest[:],
                                    start=(k == 0),
                                    stop=(k == 2 * i + 1),
                                )
                            nc.vector.reciprocal(recipt[:, i:i + 1], rps[:, 0:1])
                    # z.T = x.T @ attn accumulated per slot -> OT (SBUF);
                    # 4 d-chunks of [128,128] pack one PSUM bank
                    with tc.tile_pool(name="psav", bufs=8, space="PSUM") as psav:
                        for i in range(NSLOT):
                            avg = [
                                psav.tile([128, 512], F32, tag="av", name=f"av{i}_{g}")
                                for g in range(4)
                            ]
                            for g in range(4):
                                nc.vector.memset(avg[g][:], 0.0)
                            # d-outer so each d-chunk's PSUM completes early and
                            # its OT copy overlaps the remaining AV matmuls
                            for d in range(CC):
                                src = avg[d // 4][:, 128 * (d % 4):128 * (d % 4) + 128]
                                for k in range(2 * i + 2):
                                    nc.tensor.matmul(
                                        src,
                                        X_sb[:, k, 128 * d:128 * d + 128],
                                        attn[i][k][:],
                                        start=False,
                                        stop=(k == 2 * i + 1),
                                        skip_group_check=True,
                                    )
                                if d % 2 == 0:
                                    nc.vector.tensor_copy(
                                        OT[:, d, 128 * i:128 * i + 128], src
                                    )
                                else:
                                    nc.scalar.activation(
                                        OT[:, d, 128 * i:128 * i + 128], src, AF.Copy
                                    )

                # ======== P3: y = (z.T.T @ WvoT) * recip ========
                with (
                    tc.tile_pool(name="pwo", bufs=2) as pwo,
                    tc.tile_pool(name="psf", bufs=4, space="PSUM") as psf,
                ):
                    for e in range(ND):
                        if e == 0:
                            wo = wo0
                        else:
                            wo = pwo.tile([128, CC, 512], BF16, tag="w")
                            for q4 in range(4):
                                nc.sync.dma_start(
                                    wo[:, 4 * q4:4 * q4 + 4, :],
                                    WvoT_d[
                                        512 * q4:512 * q4 + 512, 512 * e:512 * e + 512
                                    ].rearrange("(cc p) d -> p cc d", p=128),
                                )
                        for i in range(NSLOT):
                            fps = psf.tile([128, 512], F32, tag="fps")
                            for d in range(CC):
                                nc.tensor.matmul(
                                    fps[:],
                                    OT[:, d, 128 * i:128 * i + 128],
                                    wo[:, d, :],
                                    start=(d == 0),
                                    stop=(d == CC - 1),
                                )
                            yt = stage.tile([128, 512], F32, tag="yt")
                            nc.vector.tensor_scalar_mul(
                                yt[:], fps[:], recipt[:, i:i + 1]
                            )
                            nc.sync.dma_start(
                                y_d[128 * i:128 * i + 128, 512 * e:512 * e + 512],
                                yt[:],
                            )
                pkt_cm.__exit__(None, None, None)
    nc.compile()
    return nc


def _own_blocks(h, nslot):
    return [2 * i + (i % 2 if h == 0 else 1 - i % 2) for i in range(nslot)]


def _host_prep(x, Wq, Wk, Wv, Wo, T_, C_):
    import ml_dtypes

    bf16 = ml_dtypes.bfloat16
    NSLOT = T_ // 256
    x = np.asarray(x, np.float32)
    Wq = np.asarray(Wq, np.float32)
    Wk = np.asarray(Wk, np.float32)
    Wv = np.asarray(Wv, np.float32)
    Wo = np.asarray(Wo, np.float32)
    M = np.ascontiguousarray(Wq.T @ Wk).astype(bf16)
    WvoT = np.ascontiguousarray((Wo @ Wv).T).astype(bf16)
    ones = np.ones((128, 2), bf16)
    masks = {}
    for h in (0, 1):
        own = _own_blocks(h, NSLOT)
        mb = np.zeros((NSLOT, 2, 128, 128), np.float32)
        for i, r in enumerate(own):
            ci = 2 * i + 2
            for idx in range(2):
                k = ci - 2 + idx
                s_idx = 128 * k + np.arange(128)[:, None]
                t_idx = 128 * r + np.arange(128)[None, :]
                mb[i, idx] = np.where(s_idx <= t_idx, 0.0, NEG)
        masks[h] = mb
    xN_b, xT_b = {}, {}
    for b in range(min(B, x.shape[0])):
        xN_b[b] = np.ascontiguousarray(x[b]).astype(bf16)
        xT_b[b] = np.ascontiguousarray(x[b].T).astype(bf16)
    in_maps = []
    for core in range(8):
        b, h = core // 2, core % 2
        b = b % x.shape[0]
        xN = xN_b[b]
        xT = xT_b[b]
        own = _own_blocks(h, NSLOT)
        own_cols = np.concatenate([np.arange(128 * r, 128 * r + 128) for r in own])
        xTq = np.ascontiguousarray(xT[:, own_cols])
        in_maps.append(
            {
                "xTq": xTq,
                "xT": xT,
                "xN": xN,
                "M": M,
                "WvoT": WvoT,
                "mb": masks[h],
                "ones": ones,
            }
        )
    return in_maps, masks


def kernel(x, Wq, Wk, Wv, Wo):
    from concourse.bass_utils import run_bass_kernel_spmd

    T_, C_ = T_FULL, C_FULL
    key = (T_, C_)
    if key not in _CACHE:
        _CACHE[key] = _build(T_, C_)
    nc = _CACHE[key]
    in_maps, _ = _host_prep(x, Wq, Wk, Wv, Wo, T_, C_)
    res = run_bass_kernel_spmd(nc, in_maps, list(range(8)))
    NSLOT = T_ // 256
    y = np.zeros((B, T_, C_), np.float32)
    for core in range(8):
        b, h = core // 2, core % 2
        yc = res.results[core]["y"]
        for i, r in enumerate(_own_blocks(h, NSLOT)):
            y[b, 128 * r:128 * r + 128, :] = yc[128 * i:128 * i + 128, :]
    return y


# revision 6
# speedup vs baseline: 1.0350x; 1.0021x over previous
"""Trainium2 Bass kernel for single-head causal attention (B=4, T=2048, C=2048).

Algebraic folding: y = softmax(mask(x Wq^T Wk x^T / sqrt(C))) x (Wo Wv)^T, so
the host folds the weight-only products M = Wq^T @ Wk and Wvo = Wo @ Wv once
(exact fp32), and the device computes Q' = x @ M, scores = Q' @ x^T, then
y = attn @ x @ Wvo^T. This removes the K and V projections and both K/V
AllGathers: no collectives at all, and per-core PE work drops by ~2/5.

Sharding: 8 cores = 4 batches x 2 row-interleave. The two cores of a batch
split the 16 128-row blocks so causal work balances exactly under one SPMD
program: slot i (i=0..7) owns global block r = 2i + sigma with sigma
alternating between the cores; slot i always processes 2i+2 128-col score
chunks, so the instruction stream is h-independent and the host-built mask
absorbs the causal boundary (including one fully-masked wasted chunk on the
even-sigma core).

All matmuls bf16 (separate LDWEIGHTS pipelines ahead via the PE reorder
window + FWL), fp32 PSUM accumulation. Attention runs in the transposed
domain (scores.T = [s, t]); softmax denominators via ones-matmul partition
reduction, folded into the final projection as a per-row scale. Q'.T, z.T
and x (AV stationary) stay SBUF-resident; x^T streams per 256-col chunk with
one stationary load serving every slot's scores matmul.
"""
import sys

sys.path.insert(0, "/opt/trn_rl_repo")
import numpy as np

_CACHE = {}

B = 4
T_FULL = 2048
C_FULL = 2048
NEG = -1e30


def _build(T_, C_, reps=1):
    import concourse.bacc as bacc
    import concourse.mybir as mybir
    import concourse.tile as tile

    F32 = mybir.dt.float32
    BF16 = mybir.dt.bfloat16
    AF = mybir.ActivationFunctionType
    SCALE = 1.0 / float(np.sqrt(C_FULL))

    CC = C_ // 128      # contraction 128-chunks
    ND = C_ // 512      # 512-wide d / e chunks
    TOWN = T_ // 2      # owned rows per core
    NSLOT = T_ // 256   # owned 128-row blocks (slots) per core
    NM = T_ // 256      # 256-s chunks of the full sequence

    nc = bacc.Bacc("TRN2", target_bir_lowering=False, debug=False, num_devices=8)
    xTq_d = nc.declare_dram_parameter("xTq", [C_, TOWN], BF16, isOutput=False)
    xT_d = nc.declare_dram_parameter("xT", [C_, T_], BF16, isOutput=False)
    xN_d = nc.declare_dram_parameter("xN", [T_, C_], BF16, isOutput=False)
    M_d = nc.declare_dram_parameter("M", [C_, C_], BF16, isOutput=False)
    WvoT_d = nc.declare_dram_parameter("WvoT", [C_, C_], BF16, isOutput=False)
    mb_d = nc.declare_dram_parameter("mb", [NSLOT, 2, 128, 128], F32, isOutput=False)
    ones_d = nc.declare_dram_parameter("ones", [128, 2], BF16, isOutput=False)
    y_d = nc.declare_dram_parameter("y", [TOWN, C_], F32, isOutput=True)

    with tile.TileContext(nc) as tc:
        with (
            tc.tile_pool(name="stage", bufs=4) as stage,
            tc.tile_pool(name="cst", bufs=1) as cst,
        ):
            for _rep in range(reps):
                onest = cst.tile([128, 2], BF16, tag="ones")
                recipt = cst.tile([128, NSLOT], F32, tag="recip")
                mbt = cst.tile([128, NSLOT, 2, 128], F32, tag="mb")
                QT = cst.tile([128, CC, TOWN], BF16, tag="qtr")
                OT = cst.tile([128, CC, TOWN], BF16, tag="otr")

                # pkt lives outside the P1 space so the score-chunk prefetch
                # DMAs are not WAR-gated on P1 reads; the issue points are
                # interleaved between P1's weight loads below so they don't
                # delay the chunks P1 needs first
                pkt_cm = tc.tile_pool(name="pkt", bufs=3)
                pkt = pkt_cm.__enter__()
                ktp_pre = {}

                def _prefetch_kt(m):
                    kt = pkt.tile([128, CC, 256], BF16, tag="kt", name=f"kt{m}")
                    nc.sync.dma_start(
                        kt[:],
                        xT_d[:, 256 * m:256 * m + 256].rearrange(
                            "(cc p) s -> p cc s", p=128
                        ),
                    )
                    ktp_pre[m] = kt

                wo0 = pkt.tile([128, CC, 512], BF16, tag="w0", bufs=1)

                # ======== P1: Q' = x @ M for owned rows -> QT (SBUF) ========
                with tc.tile_pool(name="px", bufs=1) as px:
                    xtq = px.tile([128, CC, TOWN], BF16, tag="xtq")
                    with (
                        tc.tile_pool(name="pwq", bufs=3) as pwq,
                        tc.tile_pool(name="ps3", bufs=8, space="PSUM") as ps3,
                    ):
                        # first weight chunk issued before xtq so the PE can
                        # start as soon as (wq cc0-3, xtq c=0) land
                        # interleave the first weight chunk with xtq so the PE
                        # streams c-chunks just behind the DMA
                        wq0 = pwq.tile([128, CC, 512], BF16, tag="w", name="wq0")
                        nc.sync.dma_start(
                            wq0[:, 0:2, :],
                            M_d[0:256, 0:512].rearrange("(cc p) d -> p cc d", p=128),
                        )
                        for c in range(CC):
                            nc.sync.dma_start(
                                xtq[:, c, :], xTq_d[128 * c:128 * c + 128, :]
                            )
                            if c in (0, 2, 6, 10):
                                q2 = {0: (2, 4), 2: (4, 8), 6: (8, 12), 10: (12, 16)}[c]
                                nc.sync.dma_start(
                                    wq0[:, q2[0]:q2[1], :],
                                    M_d[
                                        128 * q2[0]:128 * q2[1], 0:512
                                    ].rearrange("(cc p) d -> p cc d", p=128),
                                )
                        nc.sync.dma_start(onest[:], ones_d[:])
                        nc.sync.dma_start(
                            mbt[:], mb_d[:].rearrange("ns k p n -> p ns k n")
                        )
                        _prefetch_kt(0)
                        for dd in range(ND):
                            if dd == 0:
                                wq = wq0
                            else:
                                wq = pwq.tile([128, CC, 512], BF16, tag="w")
                                for q4 in range(4):
                                    nc.sync.dma_start(
                                        wq[:, 4 * q4:4 * q4 + 4, :],
                                        M_d[
                                            512 * q4:512 * q4 + 512,
                                            512 * dd:512 * dd + 512,
                                        ].rearrange("(cc p) d -> p cc d", p=128),
                                    )
                                if dd <= 2:
                                    _prefetch_kt(dd)
                            qps = [
                                ps3.tile([128, 512], F32, tag="ps3", name=f"qps{dd}_{g}")
                                for g in range(8)
                            ]
                            for c in range(CC):
                                for d4 in range(4):
                                    for tt in range(2):
                                        nc.tensor.matmul(
                                            qps[2 * d4 + tt][:],
                                            wq[:, c, 128 * d4:128 * d4 + 128],
                                            xtq[:, c, 512 * tt:512 * tt + 512],
                                            start=(c == 0),
                                            stop=(c == CC - 1),
                                        )
                            for d4 in range(4):
                                d = 4 * dd + d4
                                for tt in range(2):
                                    if tt == 0:
                                        nc.scalar.activation(
                                            QT[:, d, 512 * tt:512 * tt + 512],
                                            qps[2 * d4 + tt][:],
                                            AF.Copy,
                                        )
                                    else:
                                        nc.vector.tensor_copy(
                                            QT[:, d, 512 * tt:512 * tt + 512],
                                            qps[2 * d4 + tt][:],
                                        )

                # ======== P2: attention (transposed domain) ========
                with (
                    tc.tile_pool(name="pv", bufs=1) as pv,
                    tc.tile_pool(name="pattn", bufs=NSLOT * (NSLOT + 1)) as pattn,
                ):
                    attn = {
                        i: [
                            pattn.tile([128, 128], BF16, tag="attn", name=f"at{i}_{k}")
                            for k in range(2 * i + 2)
                        ]
                        for i in range(NSLOT)
                    }
                    # scores sweep: one stationary x^T chunk serves all slots;
                    # up to 4 slots' [128,128] scores pack one PSUM bank
                    with tc.tile_pool(name="pssc", bufs=4, space="PSUM") as pssc:
                        for m in range(NM):
                            if m < 3:
                                ktp = ktp_pre[m]
                            else:
                                ktp = pkt.tile(
                                    [128, CC, 256], BF16, tag="kt", name=f"kt{m}"
                                )
                                nc.sync.dma_start(
                                    ktp[:],
                                    xT_d[:, 256 * m:256 * m + 256].rearrange(
                                        "(cc p) s -> p cc s", p=128
                                    ),
                                )
                            nsl = NSLOT - m
                            for sub in range(2):
                                k = 2 * m + sub
                                spsg = [
                                    pssc.tile(
                                        [128, 512], F32, tag="sps", name=f"sps{k}_{g}"
                                    )
                                    for g in range((nsl + 3) // 4)
                                ]
                                for g in range((nsl + 3) // 4):
                                    nc.vector.memset(spsg[g][:], 0.0)
                                sl = lambda ii: spsg[ii // 4][
                                    :, 128 * (ii % 4):128 * (ii % 4) + 128
                                ]
                                # live slots' QT columns are contiguous, and so
                                # is each 4-slot PSUM group: one wide matmul per
                                # group instead of 4 narrow ones
                                for d in range(CC):
                                    for g in range((nsl + 3) // 4):
                                        cw = min(512, 128 * (nsl - 4 * g))
                                        nc.tensor.matmul(
                                            spsg[g][:, 0:cw],
                                            ktp[:, d, 128 * sub:128 * sub + 128],
                                            QT[
                                                :, d,
                                                128 * (m + 4 * g):128 * (m + 4 * g) + cw,
                                            ],
                                            start=False,
                                            stop=(d == CC - 1),
                                            skip_group_check=True,
                                        )
                                for ii in range(nsl):
                                    i = m + ii
                                    if i == m:
                                        nc.vector.tensor_add(
                                            sl(ii), sl(ii), mbt[:, i, sub, :]
                                        )
                                    nc.scalar.activation(
                                        attn[i][k][:], sl(ii), AF.Exp, scale=SCALE
                                    )
                    X_sb = pv.tile([128, T_ // 128, C_], BF16, tag="xsb")
                    for k in range(T_ // 128):
                        nc.sync.dma_start(X_sb[:, k, :], xN_d[128 * k:128 * k + 128, :])
                    for q4 in range(4):
                        nc.sync.dma_start(
                            wo0[:, 4 * q4:4 * q4 + 4, :],
                            WvoT_d[512 * q4:512 * q4 + 512, 0:512].rearrange(
                                "(cc p) d -> p cc d", p=128
                            ),
                        )
                    # softmax denominators -> 1/denom per owned row
                    with tc.tile_pool(name="psr", bufs=4, space="PSUM") as psr:
                        for i in range(NSLOT):
                            rps = psr.tile([128, 2], F32, tag="rps", name=f"rps{i}")
                            for k in range(2 * i + 2):
                                nc.tensor.matmul(
                                    rps[:],
                                    attn[i][k][:],
                                    onest[:],
                                    start=(k == 0),
                                    stop=(k == 2 * i + 1),
                                )
                            nc.vector.reciprocal(recipt[:, i:i + 1], rps[:, 0:1])
                    # z.T = x.T @ attn accumulated per slot -> OT (SBUF);
                    # 4 d-chunks of [128,128] pack one PSUM bank
                    with tc.tile_pool(name="psav", bufs=8, space="PSUM") as psav:
                        for i in range(NSLOT):
                            avg = [
                                psav.tile([128, 512], F32, tag="av", name=f"av{i}_{g}")
                                for g in range(4)
                            ]
                            for g in range(4):
                                nc.vector.memset(avg[g][:], 0.0)
                            # d-outer so each d-chunk's PSUM completes early and
                            # its OT copy overlaps the remaining AV matmuls
                            for d in range(CC):
                                src = avg[d // 4][:, 128 * (d % 4):128 * (d % 4) + 128]
                                for k in range(2 * i + 2):
                                    nc.tensor.matmul(
                                        src,
                                        X_sb[:, k, 128 * d:128 * d + 128],
                                        attn[i][k][:],
                                        start=False,
                                        stop=(k == 2 * i + 1),
                                        skip_group_check=True,
                                    )
                                if d % 2 == 0:
                                    nc.vector.tensor_copy(
                                        OT[:, d, 128 * i:128 * i + 128], src
                                    )
                                else:
                                    nc.scalar.activation(
                                        OT[:, d, 128 * i:128 * i + 128], src, AF.Copy
                                    )

                # ======== P3: y = (z.T.T @ WvoT) * recip ========
                with (
                    tc.tile_pool(name="pwo", bufs=2) as pwo,
                    tc.tile_pool(name="psf", bufs=4, space="PSUM") as psf,
                ):
                    for e in range(ND):
                        if e == 0:
                            wo = wo0
                        else:
                            wo = pwo.tile([128, CC, 512], BF16, tag="w")
                            for q4 in range(4):
                                nc.sync.dma_start(
                                    wo[:, 4 * q4:4 * q4 + 4, :],
                                    WvoT_d[
                                        512 * q4:512 * q4 + 512, 512 * e:512 * e + 512
                                    ].rearrange("(cc p) d -> p cc d", p=128),
                                )
                        for i in range(NSLOT):
                            fps = psf.tile([128, 512], F32, tag="fps")
                            for d in range(CC):
                                nc.tensor.matmul(
                                    fps[:],
                                    OT[:, d, 128 * i:128 * i + 128],
                                    wo[:, d, :],
                                    start=(d == 0),
                                    stop=(d == CC - 1),
                                )
                            yt = stage.tile([128, 512], F32, tag="yt")
                            nc.vector.tensor_scalar_mul(
                                yt[:], fps[:], recipt[:, i:i + 1]
                            )
                            nc.sync.dma_start(
                                y_d[128 * i:128 * i + 128, 512 * e:512 * e + 512],
                                yt[:],
                            )
                pkt_cm.__exit__(None, None, None)
    nc.compile()
    return nc


def _own_blocks(h, nslot):
    return [2 * i + (i % 2 if h == 0 else 1 - i % 2) for i in range(nslot)]


def _host_prep(x, Wq, Wk, Wv, Wo, T_, C_):
    import ml_dtypes

    bf16 = ml_dtypes.bfloat16
    NSLOT = T_ // 256
    x = np.asarray(x, np.float32)
    Wq = np.asarray(Wq, np.float32)
    Wk = np.asarray(Wk, np.float32)
    Wv = np.asarray(Wv, np.float32)
    Wo = np.asarray(Wo, np.float32)
    M = np.ascontiguousarray(Wq.T @ Wk).astype(bf16)
    WvoT = np.ascontiguousarray((Wo @ Wv).T).astype(bf16)
    ones = np.ones((128, 2), bf16)
    masks = {}
    for h in (0, 1):
        own = _own_blocks(h, NSLOT)
        mb = np.zeros((NSLOT, 2, 128, 128), np.float32)
        for i, r in enumerate(own):
            ci = 2 * i + 2
            for idx in range(2):
                k = ci - 2 + idx
                s_idx = 128 * k + np.arange(128)[:, None]
                t_idx = 128 * r + np.arange(128)[None, :]
                mb[i, idx] = np.where(s_idx <= t_idx, 0.0, NEG)
        masks[h] = mb
    xN_b, xT_b = {}, {}
    for b in range(min(B, x.shape[0])):
        xN_b[b] = np.ascontiguousarray(x[b]).astype(bf16)
        xT_b[b] = np.ascontiguousarray(x[b].T).astype(bf16)
    in_maps = []
    for core in range(8):
        b, h = core // 2, core % 2
        b = b % x.shape[0]
        xN = xN_b[b]
        xT = xT_b[b]
        own = _own_blocks(h, NSLOT)
        own_cols = np.concatenate([np.arange(128 * r, 128 * r + 128) for r in own])
        xTq = np.ascontiguousarray(xT[:, own_cols])
        in_maps.append(
            {
                "xTq": xTq,
                "xT": xT,
                "xN": xN,
                "M": M,
                "WvoT": WvoT,
                "mb": masks[h],
                "ones": ones,
            }
        )
    return in_maps, masks


def kernel(x, Wq, Wk, Wv, Wo):
    from concourse.bass_utils import run_bass_kernel_spmd

    T_, C_ = T_FULL, C_FULL
    key = (T_, C_)
    if key not in _CACHE:
        _CACHE[key] = _build(T_, C_)
    nc = _CACHE[key]
    in_maps, _ = _host_prep(x, Wq, Wk, Wv, Wo, T_, C_)
    res = run_bass_kernel_spmd(nc, in_maps, list(range(8)))
    NSLOT = T_ // 256
    y = np.zeros((B, T_, C_), np.float32)
    for core in range(8):
        b, h = core // 2, core % 2
        yc = res.results[core]["y"]
        for i, r in enumerate(_own_blocks(h, NSLOT)):
            y[b, 128 * r:128 * r + 128, :] = yc[128 * i:128 * i + 128, :]
    return y
